# revision 6
# baseline (speedup 1.0000x reference)
"""Bass/Trainium2 kernel for nn_CausalSelfAttention_15504831939088.

Multi-head attention with a key-length mask, B=2 S=2048 D=1024 H=16 DH=64,
on 8 NeuronCores.  Sharding: each core owns ONE HEAD-PAIR (heads 2c, 2c+1)
for BOTH batches.

Core idea of this revision: the QK score matmuls contract over DH=64 only
(half the PE array).  The two heads of a core sit at partitions 0..63 and
64..127, so their score matmuls carry tile_position (0,0) / (64,0) with
tile_size (64,128).  Emitted back-to-back they run CONCURRENTLY in the
PE array (row-group tiling), doubling score throughput.  The attention
loop is therefore fused over the head pair:

    per (b, q-chunk of 512) per key tile kt:
        PE : sc_h0(kt+1) ; sc_h1(kt+1)   (concurrent, 512 cols)
             pv_h0(kt)   ; pv_h1(kt)     (full-array, 512 cols each)
        ACT: pt_h0(kt+1) = exp(sc_h0 * 0.125 + mask)
        DVE: pt_h1(kt+1) = Schraudolph exp (one tensor_scalar, int16 bits)

Both heads use the leading-ones V layout [1 | 0*63 | V_h(64)]: the PV
matmul emits the softmax denominator on psum partition 0 and ctx on
partitions 64..127; normalize is recip + gpsimd partition-broadcast +
mul, staged to ctxn[h*64:...] via a small SBUF->SBUF DMA.  (Variants
with base-64 broadcast/mul corrupted trailing columns on HW.)

Projections (wq/wk/wv column shard, wo row shard) are emitted as fill
pieces between attention kt steps so the PE never idles; the 8 partial
outputs are summed per batch on the host (row-parallel reduce) with bo.
Fully-masked key tiles (kt >= ceil(valid/128)) are skipped entirely.
"""

import numpy as np

B, S, D, H = 2, 2048, 1024, 16
DH = D // H  # 64
HPC = 2      # heads per core (per batch; both batches on every core)
DHC = HPC * DH  # 128 cols per core
NST = S // 128  # 16 s-tiles (key tiles)
NKT = D // 128  # 8 contraction tiles over D
QC = 512        # query chunk (one PSUM bank wide)
NQC = S // QC   # 4 chunks per batch

_CACHE = {}
_SPEC = {"bias": (False, False, False), "full": (NST, NST),
         "nktb": (NST, NST)}


def _build(loop=1):
    """Build the SPMD Bass program + a reusable jitted runner. Cached."""
    import os as _os
    bias_spec = _SPEC["bias"]
    _key = (loop, bias_spec, _SPEC["full"], _SPEC["nktb"],
            _os.environ.get("BASS_SCBUFS", "4"),
            _os.environ.get("BASS_CXBUFS", "2"),
            _os.environ.get("BASS_PFBUFS", "2"),
            _os.environ.get("BASS_PTBUFS", "6"),
            _os.environ.get("BASS_OTBUFS", "2"),
            _os.environ.get("BASS_XF0", "0"),
            _os.environ.get("BASS_XDVE", "1"),
            _os.environ.get("BASS_PREWARM", "1"))
    if ("run", _key) in _CACHE:
        return _CACHE[("run", _key)]

    import os
    import jax
    import concourse.bass as bass
    import concourse.mybir as mybir
    import concourse.tile as tile
    from concourse import bacc, bass2jax
    from concourse.bass2jax import _bass_exec_p, partition_id_tensor
    from jax.sharding import Mesh, PartitionSpec
    from jax.experimental.shard_map import shard_map
    from contextlib import ExitStack

    f32 = mybir.dt.float32
    f32r = mybir.dt.float32r
    bf16 = mybir.dt.bfloat16
    i16 = mybir.dt.int16

    SCBUFS = int(os.environ.get("BASS_SCBUFS", "4"))
    CXBUFS = int(os.environ.get("BASS_CXBUFS", "2"))
    PFBUFS = int(os.environ.get("BASS_PFBUFS", "2"))
    PTBUFS = int(os.environ.get("BASS_PTBUFS", "6"))
    OTBUFS = int(os.environ.get("BASS_OTBUFS", "2"))
    XF0 = int(os.environ.get("BASS_XF0", "0"))  # h0 kts also on DVE (mod)
    XDVE = os.environ.get("BASS_XDVE", "1") == "1"  # h1 exp on DVE at all
    PREWARM = os.environ.get("BASS_PREWARM", "1") == "1"
    has_bq, has_bk, has_bv = bias_spec
    full_t = _SPEC["full"]     # per-batch count of fully-unmasked key tiles
    nktb = _SPEC["nktb"]       # per-batch count of key tiles to process
    # Schraudolph one-op exp on DVE: round(x*2^7/ln2 + b) bitcast to bf16.
    XA = 0.125 * (1 << 7) / float(np.log(2.0))
    XB = 16248.5

    nc = bacc.Bacc("TRN2", target_bir_lowering=False, debug=False,
                   num_devices=8)

    # host-prearranged inputs (see _shard_inputs for layouts)
    xt_d = [nc.dram_tensor(f"xt{b}", [128, NKT, S], bf16,
                           kind="ExternalInput").ap() for b in range(B)]
    wq_d = nc.dram_tensor("wq", [128, NKT * DHC], bf16,
                          kind="ExternalInput").ap()
    wk_d = nc.dram_tensor("wk", [128, NKT * DHC], bf16,
                          kind="ExternalInput").ap()
    wv_d = nc.dram_tensor("wv", [128, NKT * DHC], bf16,
                          kind="ExternalInput").ap()
    wo_d = nc.dram_tensor("wo", [DHC, D], bf16, kind="ExternalInput").ap()
    msk_d = nc.dram_tensor("msk", [128, B * NST], f32,
                           kind="ExternalInput").ap()
    y_d = [nc.dram_tensor(f"y{b}", [S, D], bf16,
                          kind="ExternalOutput").ap() for b in range(B)]
    if has_bq:
        bq_d = nc.dram_tensor("bq", [DHC], f32, kind="ExternalInput").ap()
    if has_bk:
        bk_d = nc.dram_tensor("bk", [DHC], f32, kind="ExternalInput").ap()
    if has_bv:
        bv_d = nc.dram_tensor("bv", [DHC], f32, kind="ExternalInput").ap()

    def emit_body(tc, pools):
        persist, pt_pool, rc_pool, out_pool, ps = pools
        if PREWARM:
            warm = persist.tile([128, 1], f32)
            nc.vector.memset(warm, 0.0)
            nc.scalar.activation(out=warm, in_=warm,
                                 func=mybir.ActivationFunctionType.Exp)

        # -------- persistent SBUF state + input DMAs --------
        wq_t = persist.tile([128, NKT, DHC], bf16, name="wq")
        wk_t = persist.tile([128, NKT, DHC], bf16, name="wk")
        wv_t = persist.tile([128, NKT, DHC], bf16, name="wv")
        wo_t = persist.tile([128, D], bf16, name="wo")
        mask_sb = persist.tile([128, B * NST], f32)
        nc.sync.dma_start(
            out=wq_t, in_=wq_d.rearrange("p (k c) -> p k c", c=DHC))
        nc.gpsimd.dma_start(
            out=wk_t, in_=wk_d.rearrange("p (k c) -> p k c", c=DHC))
        nc.scalar.dma_start(out=mask_sb, in_=msk_d)
        nc.scalar.dma_start(
            out=wv_t, in_=wv_d.rearrange("p (k c) -> p k c", c=DHC))
        # x loads in 128KB pieces: quarter-column-major so the first
        # projection chunk (needing cols 0:512 of all 8 k-slices) lands
        # early; round-robin the trigger queues.
        xt = [persist.tile([128, NKT, S], bf16, name=f"xt{b}")
              for b in range(B)]
        engs = [nc.sync, nc.gpsimd, nc.scalar]
        ei = 0
        for b in range(B):
            for q4 in range(4):
                for k in range(NKT):
                    engs[ei % 3].dma_start(
                        out=xt[b][:, k:k + 1, q4 * 512:(q4 + 1) * 512],
                        in_=xt_d[b][:, k:k + 1, q4 * 512:(q4 + 1) * 512])
                    ei += 1
            if b == 0:
                nc.sync.dma_start(out=wo_t, in_=wo_d)

        bias_t = {}
        if has_bq:
            t = persist.tile([128, 1], f32, name="bqs")
            nc.sync.dma_start(out=t, in_=bq_d[:, None])
            bias_t["bq"] = t
        if has_bk:
            t = persist.tile([128, 1], f32, name="bks")
            nc.sync.dma_start(out=t, in_=bk_d[:, None])
            bias_t["bk"] = t
        if has_bv:
            t = persist.tile([128, DHC], f32, name="bvs")
            nc.sync.dma_start(
                out=t, in_=bass.AP(tensor=bv_d.tensor, offset=bv_d.offset,
                                   ap=[[0, 128], [1, DHC]]))
            bias_t["bv"] = t

        qT = [persist.tile([128, S], f32r, name=f"qT{b}") for b in range(B)]
        kT = [persist.tile([128, S], f32r, name=f"kT{b}") for b in range(B)]
        # augmented V blocks [128 keys, st, 2*128] bf16, per head slot:
        # [1 | 0*63 | V_h(64)] -> PV emits denom on psum partition 0,
        # ctx on partitions 64..127
        va = [persist.tile([128, NST, HPC * 128], bf16, name=f"va{b}")
              for b in range(B)]
        for b in range(B):
            v4 = va[b].rearrange("p st (h c) -> p st h c", c=128)
            nc.vector.memset(v4[:, :, :, 0:1], 1.0)
            nc.vector.memset(v4[:, :, :, 1:DH], 0.0)
        ctxn = [persist.tile([128, S], bf16, name=f"ctxn{b}")
                for b in range(B)]

        # -------- emit helpers --------
        def emit_q(b, sc):
            pq = ps.tile([128, QC], f32, tag="pf", bufs=PFBUFS, name="pq")
            for k in range(NKT):
                nc.tensor.matmul(
                    pq, wq_t[:, k, :], xt[b][:, k, sc * 512:(sc + 1) * 512],
                    start=(k == 0), stop=(k == NKT - 1))
            if has_bq:
                nc.vector.tensor_scalar_add(
                    out=qT[b][:, sc * 512:(sc + 1) * 512], in0=pq,
                    scalar1=bias_t["bq"])
            else:
                nc.vector.tensor_copy(
                    out=qT[b][:, sc * 512:(sc + 1) * 512], in_=pq)

        def emit_k(b, sc):
            pk = ps.tile([128, QC], f32, tag="pf", bufs=PFBUFS, name="pk")
            for k in range(NKT):
                nc.tensor.matmul(
                    pk, wk_t[:, k, :], xt[b][:, k, sc * 512:(sc + 1) * 512],
                    start=(k == 0), stop=(k == NKT - 1))
            if has_bk:
                nc.scalar.tensor_scalar_add(
                    out=kT[b][:, sc * 512:(sc + 1) * 512], in0=pk,
                    scalar1=bias_t["bk"])
            else:
                nc.scalar.copy(
                    out=kT[b][:, sc * 512:(sc + 1) * 512], in_=pk)

        def emit_v(b, st):
            pv = ps.tile([128, QC], f32, tag="pf", bufs=PFBUFS, name="pv")
            for k in range(NKT):
                nc.tensor.matmul(
                    pv[:, 0:DHC], xt[b][:, k, st * 128:(st + 1) * 128],
                    wv_t[:, k, :], start=(k == 0), stop=(k == NKT - 1))
            v4 = va[b].rearrange("p st (h c) -> p st h c", c=128)
            if has_bv:
                nc.vector.tensor_add(
                    out=v4[:, st, :, DH:2 * DH],
                    in0=pv[:, 0:DHC].rearrange("p (h c) -> p h c", c=DH),
                    in1=bias_t["bv"].rearrange("p (h c) -> p h c", c=DH))
            else:
                nc.vector.tensor_copy(
                    out=v4[:, st, :, DH:2 * DH],
                    in_=pv[:, 0:DHC].rearrange("p (h c) -> p h c", c=DH))

        ot_eng = [0]

        def emit_out(b, st2):
            # two s-tiles (st2*2, st2*2+1) -> one staging tile + one DMA
            ot = out_pool.tile([128, 2, D], bf16, name="ot")
            for i in range(2):
                st = st2 * 2 + i
                for dc in range(2):
                    po = ps.tile([128, QC], f32, tag="pf", bufs=PFBUFS,
                                 name="po")
                    nc.tensor.matmul(
                        po, ctxn[b][:, st * 128:(st + 1) * 128],
                        wo_t[:, dc * 512:(dc + 1) * 512])
                    if (i + dc) % 2 == 0:
                        nc.scalar.copy(out=ot[:, i, dc * 512:(dc + 1) * 512],
                                       in_=po)
                    else:
                        nc.vector.tensor_copy(
                            out=ot[:, i, dc * 512:(dc + 1) * 512], in_=po)
            engs[ot_eng[0] % 3].dma_start(
                out=y_d[b][st2 * 256:(st2 + 1) * 256, :].rearrange(
                    "(t p) d -> p t d", p=128),
                in_=ot)
            ot_eng[0] += 1

        def emit_attn(b, qc, fills=()):
            # fused head pair for query chunk qc (512 queries)
            fills = list(fills)
            q0 = qc * QC
            nkt = nktb[b]
            cx = [ps.tile([128, QC], f32, tag="cx", bufs=CXBUFS,
                          name="cx") for _ in range(2)]

            def scores_pair(kt):
                scs = [ps.tile([128, QC], f32, tag="sc", bufs=SCBUFS,
                               name="sc") for _ in range(2)]
                # adjacent emission -> disjoint row groups run concurrently
                for h in range(2):
                    p0 = h * DH
                    nc.tensor.matmul(
                        scs[h], kT[b][p0:p0 + DH, kt * 128:(kt + 1) * 128],
                        qT[b][p0:p0 + DH, q0:q0 + QC])
                return scs

            def emit_exp(h, kt, sc):
                pt = pt_pool.tile([128, QC], bf16, name="pt")
                on_dve = kt < full_t[b] and (
                    (h == 1 and XDVE) or (h == 0 and XF0 and kt % XF0 == 1))
                if on_dve:
                    nc.vector.tensor_scalar(
                        out=pt.bitcast(i16), in0=sc,
                        scalar1=XA, scalar2=XB,
                        op0=mybir.AluOpType.mult, op1=mybir.AluOpType.add)
                else:
                    nc.scalar.activation(
                        out=pt, in_=sc,
                        func=mybir.ActivationFunctionType.Exp,
                        bias=mask_sb[:, b * NST + kt:b * NST + kt + 1],
                        scale=0.125)
                return pt

            sc_cur = scores_pair(0)
            for kt in range(nkt):
                pts = [emit_exp(h, kt, sc_cur[h]) for h in range(2)]
                if kt + 1 < nkt:
                    sc_cur = scores_pair(kt + 1)
                for _ in range(2):
                    if fills:
                        fills.pop(0)()
                for h in range(2):
                    nc.tensor.matmul(
                        cx[h], va[b][:, kt, h * 128:(h + 1) * 128], pts[h],
                        start=(kt == 0), stop=(kt == nkt - 1))
            while fills:
                fills.pop(0)()
            # normalize: reciprocal of the denominator row (partition 0),
            # gpsimd-broadcast down 64 partitions, multiply ctx rows,
            # DMA-stage into the head's ctxn partition slot.
            for h in range(2):
                rc = rc_pool.tile([1, QC], f32, tag="rc", name="rc")
                nc.vector.reciprocal(out=rc, in_=cx[h][0:1, :])
                bc64 = rc_pool.tile([DH, QC], f32, tag="bc", name="bc64")
                nc.gpsimd.partition_broadcast(bc64, rc, channels=DH)
                st64 = rc_pool.tile([DH, QC], bf16, tag="st", name="st64")
                nc.vector.tensor_mul(out=st64, in0=cx[h][DH:128, :],
                                     in1=bc64)
                nc.gpsimd.dma_start(
                    out=ctxn[b][h * DH:(h + 1) * DH, q0:q0 + QC], in_=st64)

        # -------- schedule --------
        def F(fn, *a):
            return lambda: fn(*a)

        for sc in range(4):
            emit_k(0, sc)
        emit_q(0, 0)
        for st in range(4):
            emit_v(0, st)

        emit_attn(0, 0, [F(emit_v, 0, 4), F(emit_v, 0, 5), F(emit_q, 0, 1),
                         F(emit_v, 0, 6), F(emit_q, 0, 2), F(emit_v, 0, 7),
                         F(emit_q, 0, 3)] +
                  [F(emit_v, 0, st) for st in range(8, NST)])
        emit_attn(0, 1, [F(emit_k, 1, 0), F(emit_q, 1, 0),
                         F(emit_k, 1, 1), F(emit_q, 1, 1)])
        emit_attn(0, 2, [F(emit_k, 1, 2), F(emit_q, 1, 2),
                         F(emit_k, 1, 3), F(emit_q, 1, 3)])
        emit_attn(0, 3, [F(emit_v, 1, st) for st in range(4)])
        emit_attn(1, 0, [F(emit_v, 1, st) for st in range(4, NST)])
        emit_attn(1, 1, [F(emit_out, 0, s) for s in (0, 1, 2, 3)])
        emit_attn(1, 2, [F(emit_out, 0, s) for s in (4, 5, 6, 7)] +
                  [F(emit_out, 1, 0)])
        emit_attn(1, 3, [F(emit_out, 1, s) for s in (1, 2, 3)])
        for st2 in range(4, 8):
            emit_out(1, st2)

    with tile.TileContext(nc) as tc:
        # pools are created ONCE and shared by all loop bodies: pool
        # open/close emits all-engine barriers, so per-body pools would
        # serialize iterations; shared pools let body i+1's front overlap
        # body i's tail through ordinary WAR tile dependencies.
        with ExitStack() as _pctx:
            persist = _pctx.enter_context(
                tc.tile_pool(name="persist", bufs=1))
            pt_pool = _pctx.enter_context(
                tc.tile_pool(name="pT", bufs=PTBUFS))
            rc_pool = _pctx.enter_context(tc.tile_pool(name="recip", bufs=2))
            out_pool = _pctx.enter_context(
                tc.tile_pool(name="osb", bufs=OTBUFS))
            # single PSUM pool, tag-based rotation: sc 4 + cx 2 + pf 2 = 8
            # banks of [128,512] fp32.
            ps = _pctx.enter_context(
                tc.tile_pool(name="ps", bufs=1, space="PSUM"))
            pools = (persist, pt_pool, rc_pool, out_pool, ps)
            for _ in range(loop):
                emit_body(tc, pools)

    nc.compile()
    _CACHE[("nc", loop)] = nc

    # ---- reusable PJRT runner (mirrors bass2jax.run_bass_via_pjrt) ----
    bass2jax.install_neuronx_cc_hook()
    partition_name = (nc.partition_id_tensor.name
                      if nc.partition_id_tensor else None)
    in_names, out_names, out_avals, zero_outs = [], [], [], []
    for alloc in nc.m.functions[0].allocations:
        if not isinstance(alloc, mybir.MemoryLocationSet):
            continue
        name = alloc.memorylocations[0].name
        if alloc.kind == "ExternalInput":
            if name != partition_name:
                in_names.append(name)
        elif alloc.kind == "ExternalOutput":
            out_names.append(name)
            shape = tuple(alloc.tensor_shape)
            dtype = mybir.dt.np(alloc.dtype)
            out_avals.append(jax.core.ShapedArray(shape, dtype))
            zero_outs.append(np.zeros(shape, dtype))
    n_params = len(in_names)
    in_names_all = in_names + out_names + (
        [partition_name] if partition_name else [])

    def _body(*args):
        operands = list(args)
        if partition_name is not None:
            operands.append(partition_id_tensor())
        return tuple(_bass_exec_p.bind(
            *operands, out_avals=tuple(out_avals),
            in_names=tuple(in_names_all), out_names=tuple(out_names),
            lowering_input_output_aliases=(), sim_require_finite=True,
            sim_require_nnan=True, nc=nc))

    devices = jax.devices()[:8]
    mesh = Mesh(np.asarray(devices), ("core",))
    nio = n_params + len(out_names)
    sharded = jax.jit(
        shard_map(_body, mesh=mesh, in_specs=(PartitionSpec("core"),) * nio,
                  out_specs=(PartitionSpec("core"),) * len(out_names),
                  check_rep=False),
        keep_unused=True)

    def prep(in_maps):
        concat_in = [
            np.concatenate([np.asarray(m[name]) for m in in_maps], axis=0)
            for name in in_names]
        concat_zeros = [
            np.zeros((8 * z.shape[0], *z.shape[1:]), z.dtype)
            for z in zero_outs]
        return concat_in + concat_zeros

    def run(in_maps):
        outs = sharded(*prep(in_maps))
        res = {n: np.asarray(outs[i]) for i, n in enumerate(out_names)}
        _CACHE["last_outs"] = res
        return [res[f"y{b}"].reshape(8, S, D) for b in range(B)]

    _CACHE[("run", _key)] = run
    _CACHE[("run", loop)] = run
    _CACHE[("sharded", _key)] = sharded
    _CACHE[("sharded", loop)] = sharded
    _CACHE["prep"] = prep
    return run


def _shard_inputs(x, valid_nums, Wq, bq, Wk, bk, Wv, bv, Wo, bo):
    import ml_dtypes
    bf16 = ml_dtypes.bfloat16
    _SPEC["bias"] = (bool(np.any(np.asarray(bq))),
                     bool(np.any(np.asarray(bk))),
                     bool(np.any(np.asarray(bv))))
    vn = [int(np.asarray(valid_nums)[b]) for b in range(B)]
    # key tiles with every key valid (safe for the DVE exp approximation)
    _SPEC["full"] = tuple(v // 128 for v in vn)
    # key tiles with at least one valid key (the rest are skipped)
    _SPEC["nktb"] = tuple(-(-v // 128) for v in vn)
    x = np.asarray(x, dtype=np.float32)
    idx = np.arange(S)
    # xt[b]: [128, NKT, S] with xt[p, k, s] = x[b, s, k*128+p]
    xt = [np.ascontiguousarray(
        x[b].T.reshape(NKT, 128, S).transpose(1, 0, 2)).astype(bf16)
        for b in range(B)]
    msk = np.empty((128, B * NST), np.float32)
    for b in range(B):
        m = np.where(idx < vn[b], 0.0, -1e30).astype(np.float32)
        msk[:, b * NST:(b + 1) * NST] = m.reshape(NST, 128).T
    in_maps = []
    for c in range(8):
        sl = slice(c * DHC, (c + 1) * DHC)

        def warr(w):
            # [1024, 128] col-slice -> [128, NKT*DHC] SBUF layout
            ws = np.asarray(w, np.float32)[:, sl]
            return np.ascontiguousarray(
                ws.reshape(NKT, 128, DHC).transpose(1, 0, 2).reshape(
                    128, NKT * DHC)).astype(bf16)

        m = {
            "xt0": xt[0], "xt1": xt[1],
            "wq": warr(Wq), "wk": warr(Wk), "wv": warr(Wv),
            "wo": np.ascontiguousarray(
                np.asarray(Wo, np.float32)[sl, :]).astype(bf16),
            "msk": msk,
        }
        if _SPEC["bias"][0]:
            m["bq"] = np.ascontiguousarray(np.asarray(bq, np.float32)[sl])
        if _SPEC["bias"][1]:
            m["bk"] = np.ascontiguousarray(np.asarray(bk, np.float32)[sl])
        if _SPEC["bias"][2]:
            m["bv"] = np.ascontiguousarray(np.asarray(bv, np.float32)[sl])
        in_maps.append(m)
    return in_maps


def kernel(x, valid_nums, Wq, bq, Wk, bk, Wv, bv, Wo, bo):
    in_maps = _shard_inputs(x, valid_nums, Wq, bq, Wk, bk, Wv, bv, Wo, bo)
    run = _build()
    parts = run(in_maps)  # [y0 [8,S,D], y1 [8,S,D]] bf16
    bo = np.asarray(bo, np.float32)
    out = np.empty((B, S, D), dtype=np.float32)
    for b in range(B):
        out[b] = parts[b].astype(np.float32).sum(axis=0) + bo
    return out
